# revision 6
# baseline (speedup 1.0000x reference)
"""DGN graph-attention network kernel for Trainium2 (8 NeuronCores).

Data-parallel over batch: B=16 graphs, 2 per core. Per graph, per attention
layer, the model computes s = (q@kT)@mask re-associated as s = q@(kT@mask),
a masked softmax over s, out = softmax(s)@v / sqrt(H), all fused for TRN2:

 - All fp32-precision matmuls run as float32r (full-rate PE mode, ~2^-16
   matmul error; fp32r storage rounds intermediates to ~2^-11 — validated
   end-to-end error ~5e-3 vs the f32 reference).
 - Masking is folded into the logits on the PE: s' = q@C + BMASK*(I@mask),
   so rowmax(s') = BMASK + masked-rowmax and exp(s'-m') underflows masked
   entries to exactly 0.  No elementwise mask pass is needed.
 - DVE tensor_reduce(max) gives the row max; ACT exp with per-partition
   bias computes p and the softmax denominator (accum_out) in one pass.
 - p is normalized by 1/(denom*sqrt(H)) on DVE (bf16, 4x mode), transposed
   128x128-blockwise on the PE (bf16), and contracted with v in bf16:
   outT += v_jc^T @ pT_jc, giving the transposed hidden state directly.
"""

import numpy as np
import ml_dtypes

B_FULL, N, OBS, HID, ACT_DIM = 16, 1024, 64, 128, 32
NCORES = 8
EPC = B_FULL // NCORES          # batch elements per core
NC_CH = N // 128                # 8 chunks of 128 rows
BMASK = 16384.0                 # mask fold constant (> global max logit)
INV_SQRT_H = 1.0 / np.sqrt(HID)

_CACHE = {}


def _build_program():
    import concourse.bacc as bacc
    import concourse.tile as tile
    import concourse.mybir as mybir

    F32 = mybir.dt.float32
    F32R = mybir.dt.float32r
    BF16 = mybir.dt.bfloat16
    RELU = mybir.ActivationFunctionType.Relu
    EXP = mybir.ActivationFunctionType.Exp
    COPY = mybir.ActivationFunctionType.Copy
    AMAX = mybir.AluOpType.max

    nc = bacc.Bacc("TRN2", target_bir_lowering=False, debug=False,
                   num_devices=NCORES)

    # ---------------- DRAM parameters ----------------
    xT_d = nc.dram_tensor("xT", [EPC, OBS, N], F32R, kind="ExternalInput").ap()
    mask_d = nc.dram_tensor("mask", [EPC, N, N], F32R, kind="ExternalInput").ap()
    We_d = nc.dram_tensor("We", [OBS, HID], F32R, kind="ExternalInput").ap()
    be_d = nc.dram_tensor("be_col", [HID, 1], F32, kind="ExternalInput").ap()
    Wq_d, Wk_d, Wo_d, Wv_d = [], [], [], []
    bq_d, bo_d, bk_d, bv_d = [], [], [], []
    for l in range(2):
        Wq_d.append(nc.dram_tensor(f"Wq{l}", [HID, HID], F32R, kind="ExternalInput").ap())
        Wk_d.append(nc.dram_tensor(f"Wk{l}", [HID, HID], F32R, kind="ExternalInput").ap())
        Wo_d.append(nc.dram_tensor(f"Wo{l}", [HID, HID], F32R, kind="ExternalInput").ap())
        Wv_d.append(nc.dram_tensor(f"Wv{l}", [HID, HID], BF16, kind="ExternalInput").ap())
        bq_d.append(nc.dram_tensor(f"bq{l}", [HID, 1], F32, kind="ExternalInput").ap())
        bo_d.append(nc.dram_tensor(f"bo{l}", [HID, 1], F32, kind="ExternalInput").ap())
        bk_d.append(nc.dram_tensor(f"bk{l}", [1, HID], F32R, kind="ExternalInput").ap())
        bv_d.append(nc.dram_tensor(f"bv{l}", [1, HID], BF16, kind="ExternalInput").ap())
    Wqn_d = nc.dram_tensor("Wqn", [HID, ACT_DIM], F32R, kind="ExternalInput").ap()
    bqn_d = nc.dram_tensor("bqn_row", [1, ACT_DIM], F32R, kind="ExternalInput").ap()
    Bident_d = nc.dram_tensor("Bident", [128, 128], F32R, kind="ExternalInput").ap()
    ident_d = nc.dram_tensor("ident_bf", [128, 128], BF16, kind="ExternalInput").ap()
    ones1_d = nc.dram_tensor("ones1", [1, 128], F32R, kind="ExternalInput").ap()
    ones1b_d = nc.dram_tensor("ones1_bf", [1, 128], BF16, kind="ExternalInput").ap()
    out_d = nc.dram_tensor("out", [EPC, N, ACT_DIM], F32, kind="ExternalOutput").ap()

    with tile.TileContext(nc) as tc:
        import contextlib
        ctx = contextlib.ExitStack()
        with ctx:
            consts = ctx.enter_context(tc.tile_pool(name="consts", bufs=1))
            sb = ctx.enter_context(tc.tile_pool(name="sb", bufs=2))
            masks = ctx.enter_context(tc.tile_pool(name="masks", bufs=2 * NC_CH))
            pbuf = ctx.enter_context(tc.tile_pool(name="pbuf", bufs=NC_CH + 1))
            small = ctx.enter_context(tc.tile_pool(name="small", bufs=4))
            ps_mm = ctx.enter_context(tc.tile_pool(name="ps_mm", bufs=1, space="PSUM"))
            ps_out = ctx.enter_context(tc.tile_pool(name="ps_out", bufs=1, space="PSUM"))
            ps_s = ctx.enter_context(tc.tile_pool(name="ps_s", bufs=3, space="PSUM"))
            ps_pt = ctx.enter_context(tc.tile_pool(name="ps_pt", bufs=1, space="PSUM"))

            # ------------ load constants/weights once ------------
            def ctile(shape, dt, src, tag):
                t = consts.tile(shape, dt, tag=tag)
                nc.sync.dma_start(t[:], src[:])
                return t

            We_s = ctile([OBS, HID], F32R, We_d, "We")
            be_s = ctile([HID, 1], F32, be_d, "be")
            Wq_s = [ctile([HID, HID], F32R, Wq_d[l], f"Wq{l}") for l in range(2)]
            Wk_s = [ctile([HID, HID], F32R, Wk_d[l], f"Wk{l}") for l in range(2)]
            Wo_s = [ctile([HID, HID], F32R, Wo_d[l], f"Wo{l}") for l in range(2)]
            Wv_s = [ctile([HID, HID], BF16, Wv_d[l], f"Wv{l}") for l in range(2)]
            bq_s = [ctile([HID, 1], F32, bq_d[l], f"bq{l}") for l in range(2)]
            bo_s = [ctile([HID, 1], F32, bo_d[l], f"bo{l}") for l in range(2)]
            bk_s = [ctile([1, HID], F32R, bk_d[l], f"bk{l}") for l in range(2)]
            bv_s = [ctile([1, HID], BF16, bv_d[l], f"bv{l}") for l in range(2)]
            Wqn_s = ctile([HID, ACT_DIM], F32R, Wqn_d, "Wqn")
            bqn_s = ctile([1, ACT_DIM], F32R, bqn_d, "bqn")
            Bident_s = ctile([128, 128], F32R, Bident_d, "Bident")
            ident_s = ctile([128, 128], BF16, ident_d, "ident")
            ones1_s = ctile([1, 128], F32R, ones1_d, "ones1")
            ones1b_s = ctile([1, 128], BF16, ones1b_d, "ones1b")

            for e in range(EPC):
                # ------------ loads for this element ------------
                mk = []
                for c in range(NC_CH):
                    t = masks.tile([128, N], F32R, tag="mask")
                    nc.sync.dma_start(t[:], mask_d[e, c * 128:(c + 1) * 128, :])
                    mk.append(t)
                xT_s = sb.tile([OBS, N], F32R, tag="xT")
                nc.sync.dma_start(xT_s[:], xT_d[e])

                # ------------ encoder: hT = relu(We^T xT + be) ------------
                h_ps = ps_mm.tile([128, N], F32, tag="mm")
                for half in range(2):
                    nc.tensor.matmul(h_ps[:, half * 512:(half + 1) * 512],
                                     We_s[:], xT_s[:, half * 512:(half + 1) * 512],
                                     start=True, stop=True)
                hT = sb.tile([128, N], F32R, tag="hT")
                nc.scalar.activation(hT[:], h_ps[:], RELU, bias=be_s[:])

                for l in range(2):
                    # ---- projections ----
                    hT_bf = sb.tile([128, N], BF16, tag="hTbf")
                    nc.vector.tensor_copy(hT_bf[:], hT[:])

                    q_ps = ps_mm.tile([128, N], F32, tag="mm")
                    for half in range(2):
                        nc.tensor.matmul(q_ps[:, half * 512:(half + 1) * 512],
                                         Wq_s[l][:], hT[:, half * 512:(half + 1) * 512],
                                         start=True, stop=True)
                    qT = sb.tile([128, N], F32R, tag="qT")
                    nc.scalar.activation(qT[:], q_ps[:], RELU, bias=bq_s[l][:])

                    k_ps = ps_mm.tile([128, N], F32, tag="mm")
                    for c in range(NC_CH):
                        sl = slice(c * 128, (c + 1) * 128)
                        nc.tensor.matmul(k_ps[:, sl], hT[:, sl], Wk_s[l][:],
                                         start=True, stop=False)
                        nc.tensor.matmul(k_ps[:, sl], ones1_s[:], bk_s[l][:],
                                         start=False, stop=True)
                    k_sb = sb.tile([128, N], F32R, tag="k")
                    nc.scalar.activation(k_sb[:], k_ps[:], RELU)

                    v_ps = ps_mm.tile([128, N], F32, tag="mm")
                    for c in range(NC_CH):
                        sl = slice(c * 128, (c + 1) * 128)
                        nc.tensor.matmul(v_ps[:, sl], hT_bf[:, sl], Wv_s[l][:],
                                         start=True, stop=False)
                        nc.tensor.matmul(v_ps[:, sl], ones1b_s[:], bv_s[l][:],
                                         start=False, stop=True)
                    v_sb = sb.tile([128, N], BF16, tag="v")
                    nc.scalar.activation(v_sb[:], v_ps[:], RELU)

                    # ---- C = k^T @ mask ----
                    C_ps = ps_mm.tile([128, N], F32, tag="mm")
                    for half in range(2):
                        jsl = slice(half * 512, (half + 1) * 512)
                        for c in range(NC_CH):
                            nc.tensor.matmul(C_ps[:, jsl],
                                             k_sb[:, c * 128:(c + 1) * 128],
                                             mk[c][:, jsl],
                                             start=(c == 0), stop=(c == NC_CH - 1))
                    C_sb = sb.tile([128, N], F32R, tag="C")
                    nc.scalar.activation(C_sb[:], C_ps[:], COPY)

                    # ---- scores + masked softmax per 128-row chunk ----
                    p_tiles = []
                    r_tiles = []
                    for ic in range(NC_CH):
                        qsl = slice(ic * 128, (ic + 1) * 128)
                        p_t = pbuf.tile([128, N], BF16, tag="p")
                        m_t = small.tile([128, 1], F32, tag="mrow")
                        mneg_t = small.tile([128, 1], F32, tag="mneg")
                        den_t = small.tile([128, 2], F32, tag="den")
                        halves = []
                        for half in range(2):
                            jsl = slice(half * 512, (half + 1) * 512)
                            s_ps = ps_s.tile([128, 512], F32, tag="s")
                            nc.tensor.matmul(s_ps[:], qT[:, qsl], C_sb[:, jsl],
                                             start=True, stop=False)
                            nc.tensor.matmul(s_ps[:], Bident_s[:], mk[ic][:, jsl],
                                             start=False, stop=True)
                            halves.append(s_ps)
                        mh_t = small.tile([128, 2], F32, tag="mh")
                        for half in range(2):
                            nc.vector.tensor_reduce(mh_t[:, half:half + 1], halves[half][:],
                                                    mybir.AxisListType.X, AMAX)
                        nc.vector.tensor_reduce(m_t[:], mh_t[:],
                                                mybir.AxisListType.X, AMAX)
                        nc.vector.tensor_scalar_mul(mneg_t[:], m_t[:], -1.0)
                        for half in range(2):
                            jsl = slice(half * 512, (half + 1) * 512)
                            nc.scalar.activation(p_t[:, jsl], halves[half][:], EXP,
                                                 bias=mneg_t[:],
                                                 accum_out=den_t[:, half:half + 1])
                        # rscale = 1 / (sqrt(H) * (den0 + den1))
                        dsum_t = small.tile([128, 1], F32, tag="dsum")
                        nc.vector.tensor_tensor(out=dsum_t[:], in0=den_t[:, 0:1],
                                                in1=den_t[:, 1:2],
                                                op=mybir.AluOpType.add)
                        dscaled_t = small.tile([128, 1], F32, tag="dscaled")
                        nc.vector.tensor_scalar_mul(dscaled_t[:], dsum_t[:],
                                                    float(np.sqrt(HID)))
                        r_t = small.tile([128, 1], F32, tag="rscale")
                        nc.vector.reciprocal(r_t[:], dscaled_t[:])
                        pn_t = pbuf.tile([128, N], BF16, tag="pn")
                        nc.vector.tensor_scalar_mul(pn_t[:], p_t[:], r_t[:])
                        p_tiles.append(pn_t)
                        r_tiles.append(r_t)

                    # ---- outT = sum_jc v_jc^T @ pT_jc ----
                    outT_ps = ps_out.tile([128, N], F32, tag="outT")
                    for jc in range(NC_CH):
                        jsl = slice(jc * 128, (jc + 1) * 128)
                        pT_ps = ps_pt.tile([128, N], BF16, tag="pT")
                        for ic in range(NC_CH):
                            nc.tensor.transpose(pT_ps[:, ic * 128:(ic + 1) * 128],
                                                p_tiles[ic][:, jsl], ident_s[:])
                        pT_sb = sb.tile([128, N], BF16, tag="pTsb")
                        nc.vector.tensor_copy(pT_sb[:], pT_ps[:])
                        for half in range(2):
                            hsl = slice(half * 512, (half + 1) * 512)
                            nc.tensor.matmul(outT_ps[:, hsl], v_sb[:, jsl],
                                             pT_sb[:, hsl],
                                             start=(jc == 0),
                                             stop=(jc == NC_CH - 1))
                    outT_sb = sb.tile([128, N], F32R, tag="outTsb")
                    nc.scalar.activation(outT_sb[:], outT_ps[:], COPY)

                    # ---- hT = relu(Wo^T outT + bo) ----
                    z_ps = ps_mm.tile([128, N], F32, tag="mm")
                    for half in range(2):
                        jsl = slice(half * 512, (half + 1) * 512)
                        nc.tensor.matmul(z_ps[:, jsl], Wo_s[l][:], outT_sb[:, jsl],
                                         start=True, stop=True)
                    hT = sb.tile([128, N], F32R, tag="hT")
                    nc.scalar.activation(hT[:], z_ps[:], RELU, bias=bo_s[l][:])

                # ------------ Q-net: out = h3 @ Wqn + bqn ------------
                qn_ps = ps_mm.tile([128, NC_CH * ACT_DIM], F32, tag="mm")
                for c in range(NC_CH):
                    asl = slice(c * ACT_DIM, (c + 1) * ACT_DIM)
                    nc.tensor.matmul(qn_ps[:, asl], hT[:, c * 128:(c + 1) * 128],
                                     Wqn_s[:], start=True, stop=False)
                    nc.tensor.matmul(qn_ps[:, asl], ones1_s[:], bqn_s[:],
                                     start=False, stop=True)
                qn_sb = sb.tile([128, NC_CH * ACT_DIM], F32, tag="qn")
                nc.vector.tensor_copy(qn_sb[:], qn_ps[:])
                for c in range(NC_CH):
                    nc.sync.dma_start(out_d[e, c * 128:(c + 1) * 128, :],
                                      qn_sb[:, c * ACT_DIM:(c + 1) * ACT_DIM])

    nc.compile()
    return nc


def _get_program():
    if "nc" not in _CACHE:
        _CACHE["nc"] = _build_program()
    return _CACHE["nc"]


def kernel(x, mask, We, be, att1, att2, Wqn, bqn, _trace=False):
    from concourse.bass_utils import run_bass_kernel_spmd

    nc = _get_program()
    bf = ml_dtypes.bfloat16
    f32 = np.float32

    x = np.asarray(x, dtype=f32)
    mask = np.asarray(mask, dtype=f32)
    shared = {
        "We": np.asarray(We, f32),
        "be_col": np.asarray(be, f32).reshape(HID, 1),
        "Wqn": np.asarray(Wqn, f32),
        "bqn_row": np.asarray(bqn, f32).reshape(1, ACT_DIM),
        "Bident": (BMASK * np.eye(128)).astype(f32),
        "ident_bf": np.eye(128, dtype=bf),
        "ones1": np.ones((1, 128), f32),
        "ones1_bf": np.ones((1, 128), bf),
    }
    for l, att in enumerate((att1, att2)):
        shared[f"Wq{l}"] = np.asarray(att["Wq"], f32)
        shared[f"Wk{l}"] = np.asarray(att["Wk"], f32)
        shared[f"Wo{l}"] = np.asarray(att["Wo"], f32)
        shared[f"Wv{l}"] = np.asarray(att["Wv"], f32).astype(bf)
        shared[f"bq{l}"] = np.asarray(att["bq"], f32).reshape(HID, 1)
        shared[f"bo{l}"] = np.asarray(att["bo"], f32).reshape(HID, 1)
        shared[f"bk{l}"] = np.asarray(att["bk"], f32).reshape(1, HID)
        shared[f"bv{l}"] = np.asarray(att["bv"], f32).astype(bf).reshape(1, HID)

    in_maps = []
    for c in range(NCORES):
        sl = slice(c * EPC, (c + 1) * EPC)
        m = dict(shared)
        m["xT"] = np.ascontiguousarray(x[sl].transpose(0, 2, 1))
        m["mask"] = np.ascontiguousarray(mask[sl])
        in_maps.append(m)

    res = run_bass_kernel_spmd(nc, in_maps, list(range(NCORES)), trace=_trace)
    out = np.concatenate([res.results[c]["out"] for c in range(NCORES)], axis=0)
    if _trace:
        _CACHE["last_result"] = res
    _CACHE["last_in_maps"] = in_maps
    return out.astype(np.float32)


def time_kernel(iters=10):
    """Estimate per-execution HW time by chaining `iters` NEFF executions
    inside one jitted XLA program (outputs feed the next iteration's
    donated output operands, forcing sequential execution), so the axon
    round-trip and input transfer are amortized away."""
    import time
    import jax
    import jax.numpy as jnp
    from jax.sharding import Mesh, PartitionSpec
    from jax.experimental.shard_map import shard_map
    import concourse.mybir as mybir
    from concourse import bass2jax

    nc = _get_program()
    in_maps = _CACHE["last_in_maps"]
    bass2jax.install_neuronx_cc_hook()

    partition_name = (nc.partition_id_tensor.name
                      if nc.partition_id_tensor else None)
    in_names, out_names, out_avals, zero_outs = [], [], [], []
    for alloc in nc.m.functions[0].allocations:
        if not isinstance(alloc, mybir.MemoryLocationSet):
            continue
        name = alloc.memorylocations[0].name
        if alloc.kind == "ExternalInput":
            if name != partition_name:
                in_names.append(name)
        elif alloc.kind == "ExternalOutput":
            shape = tuple(alloc.tensor_shape)
            dtype = mybir.dt.np(alloc.dtype)
            out_names.append(name)
            out_avals.append(jax.core.ShapedArray(shape, dtype))
            zero_outs.append(np.zeros(shape, dtype))
    n_params = len(in_names)
    all_names = list(in_names) + list(out_names)
    if partition_name is not None:
        all_names.append(partition_name)

    def _chain(*args):
        operands = list(args)
        if partition_name is not None:
            operands.append(bass2jax.partition_id_tensor())
        return tuple(bass2jax._bass_exec_p.bind(
            *operands,
            out_avals=tuple(out_avals),
            in_names=tuple(all_names),
            out_names=tuple(out_names),
            lowering_input_output_aliases=(),
            sim_require_finite=True,
            sim_require_nnan=True,
            nc=nc,
        ))

    devices = jax.devices()[:NCORES]
    mesh = Mesh(np.asarray(devices), ("core",))
    n_outs = len(out_names)
    sharded = jax.jit(
        shard_map(_chain, mesh=mesh,
                  in_specs=(PartitionSpec("core"),) * (n_params + n_outs),
                  out_specs=(PartitionSpec("core"),) * n_outs,
                  check_rep=False),
        donate_argnums=tuple(range(n_params, n_params + n_outs)),
        keep_unused=True,
    )
    from jax.sharding import NamedSharding
    concat_in = [
        jax.device_put(
            np.concatenate([np.asarray(in_maps[c][nm])
                            for c in range(NCORES)], axis=0),
            NamedSharding(mesh, PartitionSpec("core")))
        for nm in in_names
    ]
    jax.block_until_ready(concat_in)

    def run_once():
        czeros = [np.zeros((NCORES * z.shape[0], *z.shape[1:]), z.dtype)
                  for z in zero_outs]
        t0 = time.perf_counter()
        outs = sharded(*concat_in, *czeros)
        jax.block_until_ready(outs)
        return time.perf_counter() - t0

    run_once()            # compile + warm
    return min(run_once() for _ in range(iters))


# revision 7
# speedup vs baseline: 14.8359x; 14.8359x over previous
"""DGN graph-attention network kernel for Trainium2 (8 NeuronCores).

Data-parallel over batch: B=16 graphs, 2 per core. Per graph, per attention
layer, the model computes s = (q@kT)@mask re-associated as s = q@(kT@mask),
a masked softmax over s, out = softmax(s)@v / sqrt(H), all fused for TRN2:

 - All fp32-precision matmuls run as float32r (full-rate PE mode, ~2^-16
   matmul error; fp32r storage rounds intermediates to ~2^-11 — validated
   end-to-end error ~5e-3 vs the f32 reference).
 - Masking is folded into the logits on the PE: s' = q@C + BMASK*(I@mask),
   so rowmax(s') = BMASK + masked-rowmax and exp(s'-m') underflows masked
   entries to exactly 0.  No elementwise mask pass is needed.
 - DVE tensor_reduce(max) gives the row max; ACT exp with per-partition
   bias computes p and the softmax denominator (accum_out) in one pass.
 - p is normalized by 1/(denom*sqrt(H)) on DVE (bf16, 4x mode), transposed
   128x128-blockwise on the PE (bf16), and contracted with v in bf16:
   outT += v_jc^T @ pT_jc, giving the transposed hidden state directly.
"""

import numpy as np
import ml_dtypes

B_FULL, N, OBS, HID, ACT_DIM = 16, 1024, 64, 128, 32
NCORES = 8
EPC = B_FULL // NCORES          # batch elements per core
NC_CH = N // 128                # 8 chunks of 128 rows
BMASK = 16384.0                 # mask fold constant (> global max logit)
INV_SQRT_H = 1.0 / np.sqrt(HID)

_CACHE = {}


def _build_program(repeat=1):
    import concourse.bacc as bacc
    import concourse.tile as tile
    import concourse.mybir as mybir

    F32 = mybir.dt.float32
    F32R = mybir.dt.float32r
    BF16 = mybir.dt.bfloat16
    RELU = mybir.ActivationFunctionType.Relu
    EXP = mybir.ActivationFunctionType.Exp
    COPY = mybir.ActivationFunctionType.Copy
    AMAX = mybir.AluOpType.max

    nc = bacc.Bacc("TRN2", target_bir_lowering=False, debug=False,
                   num_devices=NCORES)

    # ---------------- DRAM parameters ----------------
    xT_d = nc.dram_tensor("xT", [EPC, OBS, N], F32R, kind="ExternalInput").ap()
    mask_d = nc.dram_tensor("mask", [EPC, N, N], F32R, kind="ExternalInput").ap()
    We_d = nc.dram_tensor("We", [OBS, HID], F32R, kind="ExternalInput").ap()
    be_d = nc.dram_tensor("be_col", [HID, 1], F32, kind="ExternalInput").ap()
    Wq_d, Wk_d, Wo_d, Wv_d = [], [], [], []
    bq_d, bo_d, bk_d, bv_d = [], [], [], []
    for l in range(2):
        Wq_d.append(nc.dram_tensor(f"Wq{l}", [HID, HID], F32R, kind="ExternalInput").ap())
        Wk_d.append(nc.dram_tensor(f"Wk{l}", [HID, HID], F32R, kind="ExternalInput").ap())
        Wo_d.append(nc.dram_tensor(f"Wo{l}", [HID, HID], F32R, kind="ExternalInput").ap())
        Wv_d.append(nc.dram_tensor(f"Wv{l}", [HID, HID], BF16, kind="ExternalInput").ap())
        bq_d.append(nc.dram_tensor(f"bq{l}", [HID, 1], F32, kind="ExternalInput").ap())
        bo_d.append(nc.dram_tensor(f"bo{l}", [HID, 1], F32, kind="ExternalInput").ap())
        bk_d.append(nc.dram_tensor(f"bk{l}", [1, HID], F32R, kind="ExternalInput").ap())
        bv_d.append(nc.dram_tensor(f"bv{l}", [1, HID], BF16, kind="ExternalInput").ap())
    Wqn_d = nc.dram_tensor("Wqn", [HID, ACT_DIM], F32R, kind="ExternalInput").ap()
    bqn_d = nc.dram_tensor("bqn_row", [1, ACT_DIM], F32R, kind="ExternalInput").ap()
    Bident_d = nc.dram_tensor("Bident", [128, 128], F32R, kind="ExternalInput").ap()
    ident_d = nc.dram_tensor("ident_bf", [128, 128], BF16, kind="ExternalInput").ap()
    ones1_d = nc.dram_tensor("ones1", [1, 128], F32R, kind="ExternalInput").ap()
    ones1b_d = nc.dram_tensor("ones1_bf", [1, 128], BF16, kind="ExternalInput").ap()
    out_d = nc.dram_tensor("out", [EPC, N, ACT_DIM], F32, kind="ExternalOutput").ap()

    with tile.TileContext(nc) as tc:
        import contextlib
        ctx = contextlib.ExitStack()
        with ctx:
            consts = ctx.enter_context(tc.tile_pool(name="consts", bufs=1))
            sb = ctx.enter_context(tc.tile_pool(name="sb", bufs=2))
            masks = ctx.enter_context(tc.tile_pool(name="masks", bufs=2 * NC_CH))
            pbuf = ctx.enter_context(tc.tile_pool(name="pbuf", bufs=NC_CH + 1))
            small = ctx.enter_context(tc.tile_pool(name="small", bufs=4))
            ps_mm = ctx.enter_context(tc.tile_pool(name="ps_mm", bufs=1, space="PSUM"))
            ps_out = ctx.enter_context(tc.tile_pool(name="ps_out", bufs=1, space="PSUM"))
            ps_s = ctx.enter_context(tc.tile_pool(name="ps_s", bufs=3, space="PSUM"))
            ps_pt = ctx.enter_context(tc.tile_pool(name="ps_pt", bufs=1, space="PSUM"))

            # ------------ load constants/weights once ------------
            def ctile(shape, dt, src, tag):
                t = consts.tile(shape, dt, tag=tag)
                nc.sync.dma_start(t[:], src[:])
                return t

            We_s = ctile([OBS, HID], F32R, We_d, "We")
            be_s = ctile([HID, 1], F32, be_d, "be")
            Wq_s = [ctile([HID, HID], F32R, Wq_d[l], f"Wq{l}") for l in range(2)]
            Wk_s = [ctile([HID, HID], F32R, Wk_d[l], f"Wk{l}") for l in range(2)]
            Wo_s = [ctile([HID, HID], F32R, Wo_d[l], f"Wo{l}") for l in range(2)]
            Wv_s = [ctile([HID, HID], BF16, Wv_d[l], f"Wv{l}") for l in range(2)]
            bq_s = [ctile([HID, 1], F32, bq_d[l], f"bq{l}") for l in range(2)]
            bo_s = [ctile([HID, 1], F32, bo_d[l], f"bo{l}") for l in range(2)]
            bk_s = [ctile([1, HID], F32R, bk_d[l], f"bk{l}") for l in range(2)]
            bv_s = [ctile([1, HID], BF16, bv_d[l], f"bv{l}") for l in range(2)]
            Wqn_s = ctile([HID, ACT_DIM], F32R, Wqn_d, "Wqn")
            bqn_s = ctile([1, ACT_DIM], F32R, bqn_d, "bqn")
            Bident_s = ctile([128, 128], F32R, Bident_d, "Bident")
            ident_s = ctile([128, 128], BF16, ident_d, "ident")
            ones1_s = ctile([1, 128], F32R, ones1_d, "ones1")
            ones1b_s = ctile([1, 128], BF16, ones1b_d, "ones1b")

            for e in [ee for _ in range(repeat) for ee in range(EPC)]:
                # ------------ loads for this element ------------
                mk = []
                for c in range(NC_CH):
                    t = masks.tile([128, N], F32R, tag="mask")
                    nc.sync.dma_start(t[:], mask_d[e, c * 128:(c + 1) * 128, :])
                    mk.append(t)
                xT_s = sb.tile([OBS, N], F32R, tag="xT")
                nc.sync.dma_start(xT_s[:], xT_d[e])

                # ------------ encoder: hT = relu(We^T xT + be) ------------
                h_ps = ps_mm.tile([128, N], F32, tag="mm")
                for half in range(2):
                    nc.tensor.matmul(h_ps[:, half * 512:(half + 1) * 512],
                                     We_s[:], xT_s[:, half * 512:(half + 1) * 512],
                                     start=True, stop=True)
                hT = sb.tile([128, N], F32R, tag="hT")
                nc.scalar.activation(hT[:], h_ps[:], RELU, bias=be_s[:])

                for l in range(2):
                    # ---- projections ----
                    hT_bf = sb.tile([128, N], BF16, tag="hTbf")
                    nc.vector.tensor_copy(hT_bf[:], hT[:])

                    q_ps = ps_mm.tile([128, N], F32, tag="mm")
                    for half in range(2):
                        nc.tensor.matmul(q_ps[:, half * 512:(half + 1) * 512],
                                         Wq_s[l][:], hT[:, half * 512:(half + 1) * 512],
                                         start=True, stop=True)
                    qT = sb.tile([128, N], F32R, tag="qT")
                    nc.scalar.activation(qT[:], q_ps[:], RELU, bias=bq_s[l][:])

                    k_ps = ps_mm.tile([128, N], F32, tag="mm")
                    for c in range(NC_CH):
                        sl = slice(c * 128, (c + 1) * 128)
                        nc.tensor.matmul(k_ps[:, sl], hT[:, sl], Wk_s[l][:],
                                         start=True, stop=False)
                        nc.tensor.matmul(k_ps[:, sl], ones1_s[:], bk_s[l][:],
                                         start=False, stop=True)
                    k_sb = sb.tile([128, N], F32R, tag="k")
                    nc.scalar.activation(k_sb[:], k_ps[:], RELU)

                    v_ps = ps_mm.tile([128, N], F32, tag="mm")
                    for c in range(NC_CH):
                        sl = slice(c * 128, (c + 1) * 128)
                        nc.tensor.matmul(v_ps[:, sl], hT_bf[:, sl], Wv_s[l][:],
                                         start=True, stop=False)
                        nc.tensor.matmul(v_ps[:, sl], ones1b_s[:], bv_s[l][:],
                                         start=False, stop=True)
                    v_sb = sb.tile([128, N], BF16, tag="v")
                    nc.scalar.activation(v_sb[:], v_ps[:], RELU)

                    # ---- C = k^T @ mask ----
                    C_ps = ps_mm.tile([128, N], F32, tag="mm")
                    for half in range(2):
                        jsl = slice(half * 512, (half + 1) * 512)
                        for c in range(NC_CH):
                            nc.tensor.matmul(C_ps[:, jsl],
                                             k_sb[:, c * 128:(c + 1) * 128],
                                             mk[c][:, jsl],
                                             start=(c == 0), stop=(c == NC_CH - 1))
                    C_sb = sb.tile([128, N], F32R, tag="C")
                    nc.scalar.activation(C_sb[:], C_ps[:], COPY)

                    # ---- scores + masked softmax per 128-row chunk ----
                    p_tiles = []
                    r_tiles = []
                    for ic in range(NC_CH):
                        qsl = slice(ic * 128, (ic + 1) * 128)
                        p_t = pbuf.tile([128, N], BF16, tag="p")
                        m_t = small.tile([128, 1], F32, tag="mrow")
                        mneg_t = small.tile([128, 1], F32, tag="mneg")
                        den_t = small.tile([128, 2], F32, tag="den")
                        halves = []
                        for half in range(2):
                            jsl = slice(half * 512, (half + 1) * 512)
                            s_ps = ps_s.tile([128, 512], F32, tag="s")
                            nc.tensor.matmul(s_ps[:], qT[:, qsl], C_sb[:, jsl],
                                             start=True, stop=False)
                            nc.tensor.matmul(s_ps[:], Bident_s[:], mk[ic][:, jsl],
                                             start=False, stop=True)
                            halves.append(s_ps)
                        mh_t = small.tile([128, 2], F32, tag="mh")
                        for half in range(2):
                            nc.vector.tensor_reduce(mh_t[:, half:half + 1], halves[half][:],
                                                    mybir.AxisListType.X, AMAX)
                        nc.vector.tensor_reduce(m_t[:], mh_t[:],
                                                mybir.AxisListType.X, AMAX)
                        nc.vector.tensor_scalar_mul(mneg_t[:], m_t[:], -1.0)
                        for half in range(2):
                            jsl = slice(half * 512, (half + 1) * 512)
                            nc.scalar.activation(p_t[:, jsl], halves[half][:], EXP,
                                                 bias=mneg_t[:],
                                                 accum_out=den_t[:, half:half + 1])
                        # rscale = 1 / (sqrt(H) * (den0 + den1))
                        dsum_t = small.tile([128, 1], F32, tag="dsum")
                        nc.vector.tensor_tensor(out=dsum_t[:], in0=den_t[:, 0:1],
                                                in1=den_t[:, 1:2],
                                                op=mybir.AluOpType.add)
                        dscaled_t = small.tile([128, 1], F32, tag="dscaled")
                        nc.vector.tensor_scalar_mul(dscaled_t[:], dsum_t[:],
                                                    float(np.sqrt(HID)))
                        r_t = small.tile([128, 1], F32, tag="rscale")
                        nc.vector.reciprocal(r_t[:], dscaled_t[:])
                        pn_t = pbuf.tile([128, N], BF16, tag="pn")
                        nc.vector.tensor_scalar_mul(pn_t[:], p_t[:], r_t[:])
                        p_tiles.append(pn_t)
                        r_tiles.append(r_t)

                    # ---- outT = sum_jc v_jc^T @ pT_jc ----
                    outT_ps = ps_out.tile([128, N], F32, tag="outT")
                    for jc in range(NC_CH):
                        jsl = slice(jc * 128, (jc + 1) * 128)
                        pT_ps = ps_pt.tile([128, N], BF16, tag="pT")
                        for ic in range(NC_CH):
                            nc.tensor.transpose(pT_ps[:, ic * 128:(ic + 1) * 128],
                                                p_tiles[ic][:, jsl], ident_s[:])
                        pT_sb = sb.tile([128, N], BF16, tag="pTsb")
                        nc.vector.tensor_copy(pT_sb[:], pT_ps[:])
                        for half in range(2):
                            hsl = slice(half * 512, (half + 1) * 512)
                            nc.tensor.matmul(outT_ps[:, hsl], v_sb[:, jsl],
                                             pT_sb[:, hsl],
                                             start=(jc == 0),
                                             stop=(jc == NC_CH - 1))
                    outT_sb = sb.tile([128, N], F32R, tag="outTsb")
                    nc.scalar.activation(outT_sb[:], outT_ps[:], COPY)

                    # ---- hT = relu(Wo^T outT + bo) ----
                    z_ps = ps_mm.tile([128, N], F32, tag="mm")
                    for half in range(2):
                        jsl = slice(half * 512, (half + 1) * 512)
                        nc.tensor.matmul(z_ps[:, jsl], Wo_s[l][:], outT_sb[:, jsl],
                                         start=True, stop=True)
                    hT = sb.tile([128, N], F32R, tag="hT")
                    nc.scalar.activation(hT[:], z_ps[:], RELU, bias=bo_s[l][:])

                # ------------ Q-net: out = h3 @ Wqn + bqn ------------
                qn_ps = ps_mm.tile([128, NC_CH * ACT_DIM], F32, tag="mm")
                for c in range(NC_CH):
                    asl = slice(c * ACT_DIM, (c + 1) * ACT_DIM)
                    nc.tensor.matmul(qn_ps[:, asl], hT[:, c * 128:(c + 1) * 128],
                                     Wqn_s[:], start=True, stop=False)
                    nc.tensor.matmul(qn_ps[:, asl], ones1_s[:], bqn_s[:],
                                     start=False, stop=True)
                qn_sb = sb.tile([128, NC_CH * ACT_DIM], F32, tag="qn")
                nc.vector.tensor_copy(qn_sb[:], qn_ps[:])
                for c in range(NC_CH):
                    nc.sync.dma_start(out_d[e, c * 128:(c + 1) * 128, :],
                                      qn_sb[:, c * ACT_DIM:(c + 1) * ACT_DIM])

    nc.compile()
    return nc


def _get_program(repeat=1):
    key = f"nc{repeat}"
    if key not in _CACHE:
        _CACHE[key] = _build_program(repeat)
    return _CACHE[key]


def kernel(x, mask, We, be, att1, att2, Wqn, bqn, _trace=False):
    from concourse.bass_utils import run_bass_kernel_spmd

    nc = _get_program()
    bf = ml_dtypes.bfloat16
    f32 = np.float32

    x = np.asarray(x, dtype=f32)
    mask = np.asarray(mask, dtype=f32)
    shared = {
        "We": np.asarray(We, f32),
        "be_col": np.asarray(be, f32).reshape(HID, 1),
        "Wqn": np.asarray(Wqn, f32),
        "bqn_row": np.asarray(bqn, f32).reshape(1, ACT_DIM),
        "Bident": (BMASK * np.eye(128)).astype(f32),
        "ident_bf": np.eye(128, dtype=bf),
        "ones1": np.ones((1, 128), f32),
        "ones1_bf": np.ones((1, 128), bf),
    }
    for l, att in enumerate((att1, att2)):
        shared[f"Wq{l}"] = np.asarray(att["Wq"], f32)
        shared[f"Wk{l}"] = np.asarray(att["Wk"], f32)
        shared[f"Wo{l}"] = np.asarray(att["Wo"], f32)
        shared[f"Wv{l}"] = np.asarray(att["Wv"], f32).astype(bf)
        shared[f"bq{l}"] = np.asarray(att["bq"], f32).reshape(HID, 1)
        shared[f"bo{l}"] = np.asarray(att["bo"], f32).reshape(HID, 1)
        shared[f"bk{l}"] = np.asarray(att["bk"], f32).reshape(1, HID)
        shared[f"bv{l}"] = np.asarray(att["bv"], f32).astype(bf).reshape(1, HID)

    in_maps = []
    for c in range(NCORES):
        sl = slice(c * EPC, (c + 1) * EPC)
        m = dict(shared)
        m["xT"] = np.ascontiguousarray(x[sl].transpose(0, 2, 1))
        m["mask"] = np.ascontiguousarray(mask[sl])
        in_maps.append(m)

    res = run_bass_kernel_spmd(nc, in_maps, list(range(NCORES)), trace=_trace)
    out = np.concatenate([res.results[c]["out"] for c in range(NCORES)], axis=0)
    if _trace:
        _CACHE["last_result"] = res
    _CACHE["last_in_maps"] = in_maps
    return out.astype(np.float32)


def time_kernel(iters=10, repeat=1):
    """Estimate per-execution HW time by chaining `iters` NEFF executions
    inside one jitted XLA program (outputs feed the next iteration's
    donated output operands, forcing sequential execution), so the axon
    round-trip and input transfer are amortized away."""
    import time
    import jax
    import jax.numpy as jnp
    from jax.sharding import Mesh, PartitionSpec
    from jax.experimental.shard_map import shard_map
    import concourse.mybir as mybir
    from concourse import bass2jax

    nc = _get_program(repeat)
    in_maps = _CACHE["last_in_maps"]
    bass2jax.install_neuronx_cc_hook()

    partition_name = (nc.partition_id_tensor.name
                      if nc.partition_id_tensor else None)
    in_names, out_names, out_avals, zero_outs = [], [], [], []
    for alloc in nc.m.functions[0].allocations:
        if not isinstance(alloc, mybir.MemoryLocationSet):
            continue
        name = alloc.memorylocations[0].name
        if alloc.kind == "ExternalInput":
            if name != partition_name:
                in_names.append(name)
        elif alloc.kind == "ExternalOutput":
            shape = tuple(alloc.tensor_shape)
            dtype = mybir.dt.np(alloc.dtype)
            out_names.append(name)
            out_avals.append(jax.core.ShapedArray(shape, dtype))
            zero_outs.append(np.zeros(shape, dtype))
    n_params = len(in_names)
    all_names = list(in_names) + list(out_names)
    if partition_name is not None:
        all_names.append(partition_name)

    def _chain(*args):
        operands = list(args)
        if partition_name is not None:
            operands.append(bass2jax.partition_id_tensor())
        return tuple(bass2jax._bass_exec_p.bind(
            *operands,
            out_avals=tuple(out_avals),
            in_names=tuple(all_names),
            out_names=tuple(out_names),
            lowering_input_output_aliases=(),
            sim_require_finite=True,
            sim_require_nnan=True,
            nc=nc,
        ))

    devices = jax.devices()[:NCORES]
    mesh = Mesh(np.asarray(devices), ("core",))
    n_outs = len(out_names)
    sharded = jax.jit(
        shard_map(_chain, mesh=mesh,
                  in_specs=(PartitionSpec("core"),) * (n_params + n_outs),
                  out_specs=(PartitionSpec("core"),) * n_outs,
                  check_rep=False),
        donate_argnums=tuple(range(n_params, n_params + n_outs)),
        keep_unused=True,
    )
    from jax.sharding import NamedSharding
    concat_in = [
        jax.device_put(
            np.concatenate([np.asarray(in_maps[c][nm])
                            for c in range(NCORES)], axis=0),
            NamedSharding(mesh, PartitionSpec("core")))
        for nm in in_names
    ]
    jax.block_until_ready(concat_in)

    def run_once():
        czeros = [np.zeros((NCORES * z.shape[0], *z.shape[1:]), z.dtype)
                  for z in zero_outs]
        t0 = time.perf_counter()
        outs = sharded(*concat_in, *czeros)
        jax.block_until_ready(outs)
        return time.perf_counter() - t0

    run_once()            # compile + warm
    return min(run_once() for _ in range(iters))


# revision 9
# speedup vs baseline: 35.5985x; 2.3995x over previous
"""DGN graph-attention network kernel for Trainium2 (8 NeuronCores).

Data-parallel over batch: B=16 graphs, 2 per core. Per graph, per attention
layer, the model computes s = (q@kT)@mask re-associated as s = q@(kT@mask),
a masked softmax over s, out = softmax(s)@v / sqrt(H), all fused for TRN2:

 - All fp32-precision matmuls run as float32r (full-rate PE mode, ~2^-16
   matmul error; fp32r storage rounds intermediates to ~2^-11 — validated
   end-to-end error ~5e-3 vs the f32 reference).
 - Masking is folded into the logits on the PE: s' = q@C + BMASK*(I@mask),
   so rowmax(s') = BMASK + masked-rowmax and exp(s'-m') underflows masked
   entries to exactly 0.  No elementwise mask pass is needed.
 - DVE tensor_reduce(max) gives the row max; ACT exp with per-partition
   bias computes p and the softmax denominator (accum_out) in one pass.
 - p is normalized by 1/(denom*sqrt(H)) on DVE (bf16, 4x mode), transposed
   128x128-blockwise on the PE (bf16), and contracted with v in bf16:
   outT += v_jc^T @ pT_jc, giving the transposed hidden state directly.
"""

import numpy as np
import ml_dtypes

B_FULL, N, OBS, HID, ACT_DIM = 16, 1024, 64, 128, 32
NCORES = 8
EPC = B_FULL // NCORES          # batch elements per core
NC_CH = N // 128                # 8 chunks of 128 rows
BMASK = 16384.0                 # mask fold constant (> global max logit)
INV_SQRT_H = 1.0 / np.sqrt(HID)

_CACHE = {}


def _build_program(repeat=1, ablate=0):
    import concourse.bacc as bacc
    import concourse.tile as tile
    import concourse.mybir as mybir

    F32 = mybir.dt.float32
    F32R = mybir.dt.float32r
    BF16 = mybir.dt.bfloat16
    RELU = mybir.ActivationFunctionType.Relu
    EXP = mybir.ActivationFunctionType.Exp
    COPY = mybir.ActivationFunctionType.Copy
    AMAX = mybir.AluOpType.max

    nc = bacc.Bacc("TRN2", target_bir_lowering=False, debug=False,
                   num_devices=NCORES)

    # ---------------- DRAM parameters ----------------
    xT_d = nc.dram_tensor("xT", [EPC, OBS, N], F32R, kind="ExternalInput").ap()
    mask_d = nc.dram_tensor("mask", [EPC, N, N], F32R, kind="ExternalInput").ap()
    We_d = nc.dram_tensor("We", [OBS, HID], F32R, kind="ExternalInput").ap()
    be_d = nc.dram_tensor("be_col", [HID, 1], F32, kind="ExternalInput").ap()
    Wq_d, Wk_d, Wo_d, Wv_d = [], [], [], []
    bq_d, bo_d, bk_d, bv_d = [], [], [], []
    for l in range(2):
        Wq_d.append(nc.dram_tensor(f"Wq{l}", [HID, HID], F32R, kind="ExternalInput").ap())
        Wk_d.append(nc.dram_tensor(f"Wk{l}", [HID, HID], F32R, kind="ExternalInput").ap())
        Wo_d.append(nc.dram_tensor(f"Wo{l}", [HID, HID], F32R, kind="ExternalInput").ap())
        Wv_d.append(nc.dram_tensor(f"Wv{l}", [HID, HID], BF16, kind="ExternalInput").ap())
        bq_d.append(nc.dram_tensor(f"bq{l}", [HID, 1], F32, kind="ExternalInput").ap())
        bo_d.append(nc.dram_tensor(f"bo{l}", [HID, 1], F32, kind="ExternalInput").ap())
        bk_d.append(nc.dram_tensor(f"bk{l}", [1, HID], F32R, kind="ExternalInput").ap())
        bv_d.append(nc.dram_tensor(f"bv{l}", [1, HID], BF16, kind="ExternalInput").ap())
    Wqn_d = nc.dram_tensor("Wqn", [HID, ACT_DIM], F32R, kind="ExternalInput").ap()
    bqn_d = nc.dram_tensor("bqn_row", [1, ACT_DIM], F32R, kind="ExternalInput").ap()
    Bident_d = nc.dram_tensor("Bident", [128, 128], F32R, kind="ExternalInput").ap()
    ident_d = nc.dram_tensor("ident_bf", [128, 128], BF16, kind="ExternalInput").ap()
    ones1_d = nc.dram_tensor("ones1", [1, 128], F32R, kind="ExternalInput").ap()
    ones1b_d = nc.dram_tensor("ones1_bf", [1, 128], BF16, kind="ExternalInput").ap()
    out_d = nc.dram_tensor("out", [EPC, N, ACT_DIM], F32, kind="ExternalOutput").ap()

    with tile.TileContext(nc) as tc:
        import contextlib
        ctx = contextlib.ExitStack()
        with ctx:
            consts = ctx.enter_context(tc.tile_pool(name="consts", bufs=1))
            sb = ctx.enter_context(tc.tile_pool(name="sb", bufs=2))
            masks = ctx.enter_context(tc.tile_pool(name="masks", bufs=2 * NC_CH))
            pbuf = ctx.enter_context(tc.tile_pool(name="pbuf", bufs=NC_CH + 1))
            small = ctx.enter_context(tc.tile_pool(name="small", bufs=4))
            ps_mm = ctx.enter_context(tc.tile_pool(name="ps_mm", bufs=1, space="PSUM"))
            ps_s = ctx.enter_context(tc.tile_pool(name="ps_s", bufs=4, space="PSUM"))
            ps_pt = ctx.enter_context(tc.tile_pool(name="ps_pt", bufs=2, space="PSUM"))

            # ------------ load constants/weights once ------------
            def ctile(shape, dt, src, tag):
                t = consts.tile(shape, dt, tag=tag)
                nc.sync.dma_start(t[:], src[:])
                return t

            We_s = ctile([OBS, HID], F32R, We_d, "We")
            be_s = ctile([HID, 1], F32, be_d, "be")
            Wq_s = [ctile([HID, HID], F32R, Wq_d[l], f"Wq{l}") for l in range(2)]
            Wk_s = [ctile([HID, HID], F32R, Wk_d[l], f"Wk{l}") for l in range(2)]
            Wo_s = [ctile([HID, HID], F32R, Wo_d[l], f"Wo{l}") for l in range(2)]
            Wv_s = [ctile([HID, HID], BF16, Wv_d[l], f"Wv{l}") for l in range(2)]
            bq_s = [ctile([HID, 1], F32, bq_d[l], f"bq{l}") for l in range(2)]
            bo_s = [ctile([HID, 1], F32, bo_d[l], f"bo{l}") for l in range(2)]
            bk_s = [ctile([1, HID], F32R, bk_d[l], f"bk{l}") for l in range(2)]
            bv_s = [ctile([1, HID], BF16, bv_d[l], f"bv{l}") for l in range(2)]
            Wqn_s = ctile([HID, ACT_DIM], F32R, Wqn_d, "Wqn")
            bqn_s = ctile([1, ACT_DIM], F32R, bqn_d, "bqn")
            Bident_s = ctile([128, 128], F32R, Bident_d, "Bident")
            ident_s = ctile([128, 128], BF16, ident_d, "ident")
            ones1_s = ctile([1, 128], F32R, ones1_d, "ones1")
            ones1b_s = ctile([1, 128], BF16, ones1b_d, "ones1b")

            for e in [ee for _ in range(repeat) for ee in range(EPC)]:
                # ------------ loads for this element ------------
                mk = []
                for c in range(NC_CH):
                    t = masks.tile([128, N], F32R, tag="mask")
                    nc.sync.dma_start(t[:], mask_d[e, c * 128:(c + 1) * 128, :])
                    mk.append(t)
                xT_s = sb.tile([OBS, N], F32R, tag="xT")
                nc.sync.dma_start(xT_s[:], xT_d[e])

                # ------------ encoder: hT = relu(We^T xT + be) ------------
                h_ps = ps_mm.tile([128, N], F32, tag="mm")
                for half in range(2):
                    nc.tensor.matmul(h_ps[:, half * 512:(half + 1) * 512],
                                     We_s[:], xT_s[:, half * 512:(half + 1) * 512],
                                     start=True, stop=True)
                hT = sb.tile([128, N], F32R, tag="hT")
                nc.scalar.activation(hT[:], h_ps[:], RELU, bias=be_s[:])

                for l in range(2):
                    # ---- projections ----
                    hT_bf = sb.tile([128, N], BF16, tag="hTbf")
                    nc.vector.tensor_copy(hT_bf[:], hT[:])

                    q_ps = ps_mm.tile([128, N], F32, tag="mm")
                    for half in range(2):
                        nc.tensor.matmul(q_ps[:, half * 512:(half + 1) * 512],
                                         Wq_s[l][:], hT[:, half * 512:(half + 1) * 512],
                                         start=True, stop=True)
                    qT = sb.tile([128, N], F32R, tag="qT")
                    nc.scalar.activation(qT[:], q_ps[:], RELU, bias=bq_s[l][:])

                    k_ps = ps_mm.tile([128, N], F32, tag="mm")
                    for c in range(NC_CH):
                        sl = slice(c * 128, (c + 1) * 128)
                        nc.tensor.matmul(k_ps[:, sl], hT[:, sl], Wk_s[l][:],
                                         start=True, stop=False)
                        nc.tensor.matmul(k_ps[:, sl], ones1_s[:], bk_s[l][:],
                                         start=False, stop=True)
                    k_sb = sb.tile([128, N], F32R, tag="k")
                    nc.scalar.activation(k_sb[:], k_ps[:], RELU)

                    v_ps = ps_mm.tile([128, N], F32, tag="mm")
                    for c in range(NC_CH):
                        sl = slice(c * 128, (c + 1) * 128)
                        nc.tensor.matmul(v_ps[:, sl], hT_bf[:, sl], Wv_s[l][:],
                                         start=True, stop=False)
                        nc.tensor.matmul(v_ps[:, sl], ones1b_s[:], bv_s[l][:],
                                         start=False, stop=True)
                    v_sb = sb.tile([128, N], BF16, tag="v")
                    nc.scalar.activation(v_sb[:], v_ps[:], RELU)

                    # ---- C = k^T @ mask ----
                    C_ps = ps_mm.tile([128, N], F32, tag="mm")
                    for half in range(2):
                        jsl = slice(half * 512, (half + 1) * 512)
                        for c in range(NC_CH):
                            nc.tensor.matmul(C_ps[:, jsl],
                                             k_sb[:, c * 128:(c + 1) * 128],
                                             mk[c][:, jsl],
                                             start=(c == 0), stop=(c == NC_CH - 1))
                    C_sb = sb.tile([128, N], F32R, tag="C")
                    nc.scalar.activation(C_sb[:], C_ps[:], COPY)

                    # ---- scores + masked softmax per 128-row chunk ----
                    p_tiles = []
                    r_tiles = []
                    if ablate == 1:
                        for ic in range(NC_CH):
                            qsl = slice(ic * 128, (ic + 1) * 128)
                            for half in range(2):
                                jsl = slice(half * 512, (half + 1) * 512)
                                s_ps = ps_s.tile([128, 512], F32, tag="s")
                                nc.tensor.matmul(s_ps[:], qT[:, qsl], C_sb[:, jsl],
                                                 start=True, stop=False)
                                nc.tensor.matmul(s_ps[:], Bident_s[:], mk[ic][:, jsl],
                                                 start=False, stop=True)
                            pn_t = pbuf.tile([128, N], BF16, tag="pn")
                            nc.gpsimd.memset(pn_t[:], 0.001)
                            p_tiles.append(pn_t)
                    for ic in range(NC_CH if ablate != 1 else 0):
                        qsl = slice(ic * 128, (ic + 1) * 128)
                        p_t = pbuf.tile([128, N], BF16, tag="p")
                        m_t = small.tile([128, 1], F32, tag="mrow")
                        mneg_t = small.tile([128, 1], F32, tag="mneg")
                        den_t = small.tile([128, 2], F32, tag="den")
                        halves = []
                        for half in range(2):
                            jsl = slice(half * 512, (half + 1) * 512)
                            s_ps = ps_s.tile([128, 512], F32, tag="s")
                            nc.tensor.matmul(s_ps[:], qT[:, qsl], C_sb[:, jsl],
                                             start=True, stop=False)
                            nc.tensor.matmul(s_ps[:], Bident_s[:], mk[ic][:, jsl],
                                             start=False, stop=True)
                            halves.append(s_ps)
                        mh_t = small.tile([128, 2], F32, tag="mh")
                        for half in range(2):
                            nc.vector.tensor_reduce(mh_t[:, half:half + 1], halves[half][:],
                                                    mybir.AxisListType.X, AMAX)
                        # mneg = -max(mh0, mh1) in one op
                        nc.vector.tensor_scalar(out=mneg_t[:], in0=mh_t[:, 0:1],
                                                scalar1=mh_t[:, 1:2], scalar2=-1.0,
                                                op0=AMAX,
                                                op1=mybir.AluOpType.mult)
                        for half in range(2):
                            jsl = slice(half * 512, (half + 1) * 512)
                            nc.scalar.activation(p_t[:, jsl], halves[half][:], EXP,
                                                 bias=mneg_t[:],
                                                 accum_out=den_t[:, half:half + 1])
                        dsum_t = small.tile([128, 1], F32, tag="dsum")
                        nc.vector.tensor_tensor(out=dsum_t[:], in0=den_t[:, 0:1],
                                                in1=den_t[:, 1:2],
                                                op=mybir.AluOpType.add)
                        r_t = small.tile([128, 1], F32, tag="rscale")
                        nc.vector.reciprocal(r_t[:], dsum_t[:])
                        # pn = (p * r) * (1/sqrt(H)) in one op
                        pn_t = pbuf.tile([128, N], BF16, tag="pn")
                        nc.vector.tensor_scalar(out=pn_t[:], in0=p_t[:], scalar1=r_t[:],
                                                scalar2=float(INV_SQRT_H),
                                                op0=mybir.AluOpType.mult,
                                                op1=mybir.AluOpType.mult)
                        p_tiles.append(pn_t)
                        r_tiles.append(r_t)

                    # ---- outT = sum_jc v_jc^T @ pT_jc ----
                    outT_ps = ps_mm.tile([128, N], F32, tag="mm")
                    for jc in range(NC_CH):
                        jsl = slice(jc * 128, (jc + 1) * 128)
                        pT_ps = ps_pt.tile([128, N], BF16, tag="pT")
                        for ic in range(NC_CH):
                            nc.tensor.transpose(pT_ps[:, ic * 128:(ic + 1) * 128],
                                                p_tiles[ic][:, jsl], ident_s[:])
                        pT_sb = sb.tile([128, N], BF16, tag="pTsb")
                        nc.vector.tensor_copy(pT_sb[:], pT_ps[:])
                        for half in range(2):
                            hsl = slice(half * 512, (half + 1) * 512)
                            nc.tensor.matmul(outT_ps[:, hsl], v_sb[:, jsl],
                                             pT_sb[:, hsl],
                                             start=(jc == 0),
                                             stop=(jc == NC_CH - 1))
                    outT_sb = sb.tile([128, N], F32R, tag="outTsb")
                    nc.scalar.activation(outT_sb[:], outT_ps[:], COPY)

                    # ---- hT = relu(Wo^T outT + bo) ----
                    z_ps = ps_mm.tile([128, N], F32, tag="mm")
                    for half in range(2):
                        jsl = slice(half * 512, (half + 1) * 512)
                        nc.tensor.matmul(z_ps[:, jsl], Wo_s[l][:], outT_sb[:, jsl],
                                         start=True, stop=True)
                    hT = sb.tile([128, N], F32R, tag="hT")
                    nc.scalar.activation(hT[:], z_ps[:], RELU, bias=bo_s[l][:])

                # ------------ Q-net: out = h3 @ Wqn + bqn ------------
                qn_ps = ps_mm.tile([128, NC_CH * ACT_DIM], F32, tag="mm")
                for c in range(NC_CH):
                    asl = slice(c * ACT_DIM, (c + 1) * ACT_DIM)
                    nc.tensor.matmul(qn_ps[:, asl], hT[:, c * 128:(c + 1) * 128],
                                     Wqn_s[:], start=True, stop=False)
                    nc.tensor.matmul(qn_ps[:, asl], ones1_s[:], bqn_s[:],
                                     start=False, stop=True)
                qn_sb = sb.tile([128, NC_CH * ACT_DIM], F32, tag="qn")
                nc.vector.tensor_copy(qn_sb[:], qn_ps[:])
                for c in range(NC_CH):
                    nc.sync.dma_start(out_d[e, c * 128:(c + 1) * 128, :],
                                      qn_sb[:, c * ACT_DIM:(c + 1) * ACT_DIM])

    nc.compile()
    return nc


def _get_program(repeat=1, ablate=0):
    key = f"nc{repeat}_{ablate}"
    if key not in _CACHE:
        _CACHE[key] = _build_program(repeat, ablate)
    return _CACHE[key]


def kernel(x, mask, We, be, att1, att2, Wqn, bqn, _trace=False):
    from concourse.bass_utils import run_bass_kernel_spmd

    nc = _get_program()
    bf = ml_dtypes.bfloat16
    f32 = np.float32

    x = np.asarray(x, dtype=f32)
    mask = np.asarray(mask, dtype=f32)
    shared = {
        "We": np.asarray(We, f32),
        "be_col": np.asarray(be, f32).reshape(HID, 1),
        "Wqn": np.asarray(Wqn, f32),
        "bqn_row": np.asarray(bqn, f32).reshape(1, ACT_DIM),
        "Bident": (BMASK * np.eye(128)).astype(f32),
        "ident_bf": np.eye(128, dtype=bf),
        "ones1": np.ones((1, 128), f32),
        "ones1_bf": np.ones((1, 128), bf),
    }
    for l, att in enumerate((att1, att2)):
        shared[f"Wq{l}"] = np.asarray(att["Wq"], f32)
        shared[f"Wk{l}"] = np.asarray(att["Wk"], f32)
        shared[f"Wo{l}"] = np.asarray(att["Wo"], f32)
        shared[f"Wv{l}"] = np.asarray(att["Wv"], f32).astype(bf)
        shared[f"bq{l}"] = np.asarray(att["bq"], f32).reshape(HID, 1)
        shared[f"bo{l}"] = np.asarray(att["bo"], f32).reshape(HID, 1)
        shared[f"bk{l}"] = np.asarray(att["bk"], f32).reshape(1, HID)
        shared[f"bv{l}"] = np.asarray(att["bv"], f32).astype(bf).reshape(1, HID)

    in_maps = []
    for c in range(NCORES):
        sl = slice(c * EPC, (c + 1) * EPC)
        m = dict(shared)
        m["xT"] = np.ascontiguousarray(x[sl].transpose(0, 2, 1))
        m["mask"] = np.ascontiguousarray(mask[sl])
        in_maps.append(m)

    res = run_bass_kernel_spmd(nc, in_maps, list(range(NCORES)), trace=_trace)
    out = np.concatenate([res.results[c]["out"] for c in range(NCORES)], axis=0)
    if _trace:
        _CACHE["last_result"] = res
    _CACHE["last_in_maps"] = in_maps
    return out.astype(np.float32)


def time_kernel(iters=10, repeat=1, ablate=0):
    """Estimate per-execution HW time by chaining `iters` NEFF executions
    inside one jitted XLA program (outputs feed the next iteration's
    donated output operands, forcing sequential execution), so the axon
    round-trip and input transfer are amortized away."""
    import time
    import jax
    import jax.numpy as jnp
    from jax.sharding import Mesh, PartitionSpec
    from jax.experimental.shard_map import shard_map
    import concourse.mybir as mybir
    from concourse import bass2jax

    nc = _get_program(repeat, ablate)
    in_maps = _CACHE["last_in_maps"]
    bass2jax.install_neuronx_cc_hook()

    partition_name = (nc.partition_id_tensor.name
                      if nc.partition_id_tensor else None)
    in_names, out_names, out_avals, zero_outs = [], [], [], []
    for alloc in nc.m.functions[0].allocations:
        if not isinstance(alloc, mybir.MemoryLocationSet):
            continue
        name = alloc.memorylocations[0].name
        if alloc.kind == "ExternalInput":
            if name != partition_name:
                in_names.append(name)
        elif alloc.kind == "ExternalOutput":
            shape = tuple(alloc.tensor_shape)
            dtype = mybir.dt.np(alloc.dtype)
            out_names.append(name)
            out_avals.append(jax.core.ShapedArray(shape, dtype))
            zero_outs.append(np.zeros(shape, dtype))
    n_params = len(in_names)
    all_names = list(in_names) + list(out_names)
    if partition_name is not None:
        all_names.append(partition_name)

    def _chain(*args):
        operands = list(args)
        if partition_name is not None:
            operands.append(bass2jax.partition_id_tensor())
        return tuple(bass2jax._bass_exec_p.bind(
            *operands,
            out_avals=tuple(out_avals),
            in_names=tuple(all_names),
            out_names=tuple(out_names),
            lowering_input_output_aliases=(),
            sim_require_finite=True,
            sim_require_nnan=True,
            nc=nc,
        ))

    devices = jax.devices()[:NCORES]
    mesh = Mesh(np.asarray(devices), ("core",))
    n_outs = len(out_names)
    sharded = jax.jit(
        shard_map(_chain, mesh=mesh,
                  in_specs=(PartitionSpec("core"),) * (n_params + n_outs),
                  out_specs=(PartitionSpec("core"),) * n_outs,
                  check_rep=False),
        donate_argnums=tuple(range(n_params, n_params + n_outs)),
        keep_unused=True,
    )
    from jax.sharding import NamedSharding
    concat_in = [
        jax.device_put(
            np.concatenate([np.asarray(in_maps[c][nm])
                            for c in range(NCORES)], axis=0),
            NamedSharding(mesh, PartitionSpec("core")))
        for nm in in_names
    ]
    jax.block_until_ready(concat_in)

    def run_once():
        czeros = [np.zeros((NCORES * z.shape[0], *z.shape[1:]), z.dtype)
                  for z in zero_outs]
        t0 = time.perf_counter()
        outs = sharded(*concat_in, *czeros)
        jax.block_until_ready(outs)
        return time.perf_counter() - t0

    run_once()            # compile + warm
    return min(run_once() for _ in range(iters))
